# revision 6
# baseline (speedup 1.0000x reference)
"""Trainium2 Bass kernel for nn_DesignerNetwork (GNN message passing, 8-core data parallel).

Strategy:
- Shard batch B=512 across 8 NeuronCores (64 per core); replicate weights.
- Reformulate the reference's O(N^2) nested scans as 64 sequential "source steps"
  per sweep: when node j's rho finalizes, push one batched GRU update into every
  DAG successor's accumulator state (compile-time sparse: the adjacency is known
  when the kernel is traced, so only real edges are processed).
- The per-batch `has` mask is folded into the z-gate pre-activation (a -BIG
  additive term saturates the gate so masked columns keep their state exactly);
  the z-gate is sign-flipped in the weights so sigmoid yields z' = 1-z directly.
- The alpha_f/alpha_b aggregation loops are virtual 65th targets of each sweep.
- Layout: state-dim on partitions, (target, batch) on the free axis. Biases ride
  constant-one rows of the matmul inputs; rho/x contributions enter PSUM via
  stride-0 broadcast matmuls so no staging copies are needed.
"""
import sys

sys.path.insert(0, "/opt/trn_rl_repo")

import numpy as np
from concourse import bass, bacc, tile
from concourse import mybir
from concourse.bass_utils import run_bass_kernel_spmd

dt = mybir.dt
Alu = mybir.AluOpType
Act = mybir.ActivationFunctionType

N, I, O = 64, 8, 8
S = 60          # S_PHI == S_RHO
SEX = 10
NROLES, NACT = 7, 5
NCORES = 8
BSH = 64        # batch per core
NT = N + 1      # targets incl virtual aggregation node
BIG = 50.0
CHUNK = 8       # target blocks per psum chunk (8*64 = 512 cols = 1 psum bank)

# compute dtype for SBUF state / matmul operands ("float32" or "bfloat16")
COMPUTE_DTYPE = "bfloat16"

_cache = {}


# ----------------------------------------------------------------------------
# host-side preprocessing
# ----------------------------------------------------------------------------

def _edge_schedule(adj):
    """Per-step target lists. fwd[j] = targets i>j with edge j->i (+virtual 64).
    bwd is indexed by step order (j descending)."""
    a = np.triu(np.asarray(adj), 1) != 0
    fwd = []
    for j in range(N):
        tgts = [i for i in range(j + 1, N) if a[j, i]]
        if j >= N - O:
            tgts.append(N)
        fwd.append(tgts)
    bwd = []
    for j in range(N - 1, -1, -1):
        tgts = [i for i in range(j) if a[i, j]]
        if j < I:
            tgts.append(N)
        bwd.append((j, tgts))
    return fwd, bwd


def _prep_weights(inp, np_dt):
    """Build the device-side lhsT / rhs weight tiles (numpy, host)."""
    w = {}

    def gru_tiles(p, Wih, Whh, bih, bhh):
        b = bih + bhh
        WXT = np.zeros((62, 124), np.float32)
        WXT[0:60, 0:60] = Wih[0:60].T
        WXT[0:60, 64:124] = -Wih[60:120].T
        WXT[60, 0:60] = b[0:60]
        WXT[60, 64:124] = -b[60:120]
        WXT[61, 64:124] = 1.0          # picks up the -BIG*(1-has) mask row
        w["wxt_" + p] = WXT
        WHT = np.zeros((60, 124), np.float32)   # z' outputs live at 64:124
        WHT[:, 0:60] = Whh[0:60].T
        WHT[:, 64:124] = -Whh[60:120].T
        w["wht_" + p] = WHT
        WHN = np.concatenate([Whh[120:180].T, bhh[None, 120:180]], 0)  # [61,60]
        w["whn_" + p] = WHN
        WIN = np.concatenate([Wih[120:180].T, bih[None, 120:180]], 0)  # [61,60]
        w["win_" + p] = WIN

    gru_tiles("f", inp["Wih_f"], inp["Whh_f"], inp["bih_f"], inp["bhh_f"])
    gru_tiles("b", inp["Wih_b"], inp["Whh_b"], inp["bih_b"], inp["bhh_b"])

    for p, W, bvec in (("f", inp["Wf"], inp["bf"]), ("b", inp["Wb"], inp["bb"])):
        w["wfh_" + p] = np.concatenate([W[:, 0:60].T, bvec[None, :]], 0)  # [61,60]
        w["wfz_" + p] = np.ascontiguousarray(W[:, 60:80].T)               # [20,60]

    WOV = np.zeros((125, 6), np.float32)
    WOV[0:60, 0:5] = inp["Wa"].T[0:60]
    WOV[64:124, 0:5] = inp["Wa"].T[60:120]
    WOV[0:60, 5] = inp["Wc"][0][0:60]
    WOV[64:124, 5] = inp["Wc"][0][60:120]
    WOV[124, 0:5] = inp["ba"]
    WOV[124, 5] = inp["bc"][0]
    w["wov"] = WOV
    w["wu7a"] = np.concatenate([inp["Wu"][:, 0:60].T, inp["bu"][None, :]], 0)
    w["wu7b"] = np.ascontiguousarray(inp["Wu"][:, 60:120].T)
    return {k: v.astype(np_dt) for k, v in w.items()}


def _prep_core_inputs(z, dz, has, core, np_dt):
    b0 = core * BSH
    zs = z[b0:b0 + BSH]            # [64, 64, 10]
    dzs = dz[b0:b0 + BSH]
    hs = has[b0:b0 + BSH]          # [64, 64]
    zdz = np.concatenate(
        [zs.transpose(2, 1, 0).reshape(SEX, N * BSH),
         dzs.transpose(2, 1, 0).reshape(SEX, N * BSH)], 0)   # [20, 4096]
    hasneg = (-BIG * (1.0 - hs.T)).reshape(1, N * BSH)       # [1, 4096]
    return {
        "zdz": zdz.astype(np_dt),
        "hasneg": hasneg.astype(np_dt),
        "hasmat": hs.astype(np.float32),                     # [64, 64] (b, n)
    }


# ----------------------------------------------------------------------------
# device program
# ----------------------------------------------------------------------------

def _emit_sweep(nc, pools, tiles, sched, pfx, tanh_rho):
    """One sweep (fwd or bwd). sched: list of (j, targets)."""
    cdt = tiles["cdt"]
    H = tiles["H_" + pfx]
    RHO = tiles["RHO_" + pfx]
    ZDZ = tiles["ZDZ"]
    WXT, WHT, WHN, WIN = (tiles[k + "_" + pfx] for k in ("wxt", "wht", "whn", "win"))
    WFH, WFZ = tiles["wfh_" + pfx], tiles["wfz_" + pfx]
    p_rho, p_inn, p_rz, p_hn, p_work = (pools[k] for k in (
        "psum_rho", "psum_inn", "psum_rz", "psum_hn", "work"))
    for (j, tgts) in sched:
        jb = slice(j * BSH, (j + 1) * BSH)
        # --- finalize rho_j from its accumulated state ---
        ps_rho = p_rho.tile([S, BSH], dt.float32, tag="ps_rho")
        nc.tensor.matmul(ps_rho[:], WFH[:], H[0:61, jb], start=True, stop=False)
        nc.tensor.matmul(ps_rho[:], WFZ[:], ZDZ[:, jb], start=False, stop=True)
        if tanh_rho:
            nc.scalar.activation(RHO[0:60, jb], ps_rho[:], Act.Tanh)
        else:
            nc.vector.tensor_copy(RHO[0:60, jb], ps_rho[:])
        if not tgts:
            continue
        # --- shared input-side n-gate projection for this step ---
        ps_inn = p_inn.tile([S, BSH], dt.float32, tag="ps_inn")
        nc.tensor.matmul(ps_inn[:], WIN[:], RHO[0:61, jb], start=True, stop=True)
        # --- push one GRU update into every successor, in <=8-block chunks ---
        for c0 in range(0, len(tgts), CHUNK):
            blk = tgts[c0:c0 + CHUNK]
            E = len(blk)
            n = E * BSH
            ps_rz = p_rz.tile([124, CHUNK * BSH], dt.float32, tag="ps_rz")
            ps_hn = p_hn.tile([S, CHUNK * BSH], dt.float32, tag="ps_hn")
            # x-side (broadcast rho_j across the chunk) + bias + has-mask rows
            rhs_x = RHO[0:62, jb].unsqueeze(1).broadcast_to((62, E, BSH))
            nc.tensor.matmul(
                ps_rz[:, 0:n].rearrange("p (e c) -> p e c", e=E),
                WXT[:], rhs_x, start=True, stop=True)
            # h-side per successor block
            for k, tgt in enumerate(blk):
                sl = slice(k * BSH, (k + 1) * BSH)
                tb = slice(tgt * BSH, (tgt + 1) * BSH)
                nc.tensor.matmul(ps_rz[:, sl], WHT[:], H[0:60, tb],
                                 start=False, stop=False, skip_group_check=True)
            for k, tgt in enumerate(blk):
                sl = slice(k * BSH, (k + 1) * BSH)
                tb = slice(tgt * BSH, (tgt + 1) * BSH)
                nc.tensor.matmul(ps_hn[:, sl], WHN[:], H[0:61, tb],
                                 start=True, stop=True)
            # gates (r and z' to separate base-0 tiles: DVE tensor_tensor
            # requires equal base partitions for SBUF operand pairs)
            rr = p_work.tile([S, CHUNK * BSH], cdt, tag="rr")
            nc.scalar.activation(rr[:, 0:n], ps_rz[0:60, 0:n], Act.Sigmoid)
            zz = p_work.tile([S, CHUNK * BSH], cdt, tag="zz")
            nc.scalar.activation(zz[:, 0:n], ps_rz[64:124, 0:n], Act.Sigmoid)
            t1 = p_work.tile([S, CHUNK * BSH], cdt, tag="t1")
            nc.vector.tensor_tensor(t1[:, 0:n], rr[:, 0:n], ps_hn[:, 0:n],
                                    Alu.mult)
            u = p_work.tile([S, CHUNK * BSH], cdt, tag="u")
            nc.vector.tensor_tensor(
                u[:, 0:n].rearrange("p (e c) -> p e c", e=E),
                t1[:, 0:n].rearrange("p (e c) -> p e c", e=E),
                ps_inn[:].unsqueeze(1).broadcast_to((S, E, BSH)),
                Alu.add)
            nn_t = p_work.tile([S, CHUNK * BSH], cdt, tag="nn")
            nc.scalar.activation(nn_t[:, 0:n], u[:, 0:n], Act.Tanh)
            dd = p_work.tile([S, CHUNK * BSH], cdt, tag="dd")
            for k, tgt in enumerate(blk):
                sl = slice(k * BSH, (k + 1) * BSH)
                tb = slice(tgt * BSH, (tgt + 1) * BSH)
                nc.gpsimd.tensor_sub(dd[:, sl], nn_t[:, sl], H[0:60, tb])
            ee = p_work.tile([S, CHUNK * BSH], cdt, tag="ee")
            nc.vector.tensor_tensor(ee[:, 0:n], zz[:, 0:n], dd[:, 0:n],
                                    Alu.mult)
            for k, tgt in enumerate(blk):
                sl = slice(k * BSH, (k + 1) * BSH)
                tb = slice(tgt * BSH, (tgt + 1) * BSH)
                nc.vector.tensor_add(H[0:60, tb], H[0:60, tb], ee[:, sl])


def _emit_program(nc, tc, adj):
    cdt = dt.bfloat16 if COMPUTE_DTYPE == "bfloat16" else dt.float32
    fwd, bwd = _edge_schedule(adj)

    win = {}
    def din(name, shape, ddt=None):
        win[name] = nc.dram_tensor(name, list(shape), ddt or cdt,
                                   kind="ExternalInput").ap()
        return win[name]

    for p in ("f", "b"):
        din("wxt_" + p, (62, 124)); din("wht_" + p, (60, 124))
        din("whn_" + p, (61, 60)); din("win_" + p, (61, 60))
        din("wfh_" + p, (61, 60)); din("wfz_" + p, (20, 60))
    din("wov", (125, 6)); din("wu7a", (61, 7)); din("wu7b", (60, 7))
    din("zdz", (2 * SEX, N * BSH)); din("hasneg", (1, N * BSH))
    din("hasmat", (BSH, N), dt.float32)
    d_instr = nc.dram_tensor("out_instr", [BSH, NACT], dt.float32,
                             kind="ExternalOutput").ap()
    d_role = nc.dram_tensor("out_role", [BSH, NROLES, N], dt.float32,
                            kind="ExternalOutput").ap()
    d_value = nc.dram_tensor("out_value", [BSH, 1], dt.float32,
                             kind="ExternalOutput").ap()

    from contextlib import ExitStack
    with ExitStack() as ctx:
        pools = {
            "const": ctx.enter_context(tc.tile_pool(name="const", bufs=1)),
            "work": ctx.enter_context(tc.tile_pool(name="work", bufs=3)),
            "psum_rho": ctx.enter_context(
                tc.tile_pool(name="psum_rho", bufs=1, space="PSUM")),
            "psum_inn": ctx.enter_context(
                tc.tile_pool(name="psum_inn", bufs=2, space="PSUM")),
            "psum_rz": ctx.enter_context(
                tc.tile_pool(name="psum_rz", bufs=2, space="PSUM")),
            "psum_hn": ctx.enter_context(
                tc.tile_pool(name="psum_hn", bufs=2, space="PSUM")),
            "psum_heads": ctx.enter_context(
                tc.tile_pool(name="psum_heads", bufs=1, space="PSUM")),
        }
        pc = pools["const"]
        tiles = {"cdt": cdt}
        # persistent state + inputs
        for p in ("f", "b"):
            Ht = pc.tile([61, NT * BSH], cdt, tag="H_" + p)
            nc.vector.memset(Ht[:], 1.0)
            nc.vector.memset(Ht[0:60, :], 0.0)
            tiles["H_" + p] = Ht
            Rt = pc.tile([62, N * BSH], cdt, tag="RHO_" + p)
            nc.vector.memset(Rt[:], 1.0)
            nc.sync.dma_start(Rt[61:62, :], win["hasneg"])
            tiles["RHO_" + p] = Rt
        zt = pc.tile([2 * SEX, N * BSH], cdt, tag="ZDZ")
        nc.sync.dma_start(zt[:], win["zdz"])
        tiles["ZDZ"] = zt
        hm = pc.tile([BSH, N], dt.float32, tag="hasmat")
        nc.sync.dma_start(hm[:], win["hasmat"])
        tiles["hasmat"] = hm
        for name, ap in win.items():
            if name in ("zdz", "hasneg", "hasmat"):
                continue
            wt = pc.tile(list(ap.shape), cdt, tag=name)
            nc.sync.dma_start(wt[:], ap)
            tiles[name] = wt

        # interleave the two independent sweeps step by step
        fwd_sched = [(j, fwd[j]) for j in range(N)]
        bwd_sched = bwd
        for k in range(N):
            _emit_sweep(nc, pools, tiles, [fwd_sched[k]], "f", True)
            _emit_sweep(nc, pools, tiles, [bwd_sched[k]], "b", False)

        # ---------------- heads ----------------
        Hf, Hb = tiles["H_f"], tiles["H_b"]
        RHOf, RHOb = tiles["RHO_f"], tiles["RHO_b"]
        vb = slice(N * BSH, NT * BSH)
        ab = pc.tile([125, BSH], cdt, tag="ab")
        nc.vector.memset(ab[:], 1.0)
        nc.vector.tensor_copy(ab[0:60, :], Hf[0:60, vb])
        nc.vector.tensor_copy(ab[64:124, :], Hb[0:60, vb])
        ps_ov = pools["psum_heads"].tile([BSH, 6], dt.float32, tag="ps_heads")
        nc.tensor.matmul(ps_ov[:], ab[:], tiles["wov"][:], start=True, stop=True)
        ov = pc.tile([BSH, 6], dt.float32, tag="ov")
        nc.vector.tensor_copy(ov[:], ps_ov[:])
        nc.sync.dma_start(d_value, ov[:, 5:6])
        # instr softmax (free axis = 5 actions)
        mx = pc.tile([BSH, 1], dt.float32, tag="mx")
        nc.vector.tensor_reduce(mx[:], ov[:, 0:5], axis=mybir.AxisListType.X,
                                op=Alu.max, negate=True)
        ex = pc.tile([BSH, NACT], dt.float32, tag="ex")
        nc.scalar.activation(ex[:], ov[:, 0:5], Act.Exp, bias=mx[:])
        sm = pc.tile([BSH, 1], dt.float32, tag="sm")
        nc.vector.tensor_reduce(sm[:], ex[:], axis=mybir.AxisListType.X,
                                op=Alu.add)
        rs = pc.tile([BSH, 1], dt.float32, tag="rs")
        nc.vector.reciprocal(rs[:], sm[:])
        ip = pc.tile([BSH, NACT], dt.float32, tag="ip")
        nc.vector.tensor_scalar_mul(ip[:], ex[:], rs[:])
        nc.sync.dma_start(d_instr, ip[:])
        # psi: [64b, (r, n)] accumulated per node
        ps_psi = pools["psum_heads"].tile([BSH, NROLES * N], dt.float32,
                                          tag="ps_heads")
        psi3 = ps_psi[:].rearrange("p (r n) -> p r n", n=N)
        for node in range(N):
            nb = slice(node * BSH, (node + 1) * BSH)
            nc.tensor.matmul(psi3[:, :, node], RHOf[0:61, nb], tiles["wu7a"][:],
                             start=True, stop=False)
            nc.tensor.matmul(psi3[:, :, node], RHOb[0:60, nb], tiles["wu7b"][:],
                             start=False, stop=True)
        # mask: softmax input == (psi + 60) * has  (softmax shift-invariance)
        psi_s = pc.tile([BSH, NROLES * N], dt.float32, tag="psi_s")
        psi_sv = psi_s[:].rearrange("p (r n) -> p r n", n=N)
        nc.vector.scalar_tensor_tensor(
            psi_sv, psi3, 60.0,
            hm[:].unsqueeze(1).broadcast_to((BSH, NROLES, N)),
            op0=Alu.add, op1=Alu.mult)
        mxn = pc.tile([BSH, NROLES], dt.float32, tag="mxn")
        nc.vector.tensor_reduce(mxn[:], psi_sv, axis=mybir.AxisListType.X,
                                op=Alu.max, negate=True)
        exn = pc.tile([BSH, NROLES * N], dt.float32, tag="exn")
        exn_v = exn[:].rearrange("p (r n) -> p r n", n=N)
        nc.vector.tensor_tensor(exn_v, psi_sv,
                                mxn[:].unsqueeze(2).broadcast_to((BSH, NROLES, N)),
                                Alu.add)
        nc.scalar.activation(exn[:], exn[:], Act.Exp)
        smn = pc.tile([BSH, NROLES], dt.float32, tag="smn")
        nc.vector.tensor_reduce(smn[:], exn_v, axis=mybir.AxisListType.X,
                                op=Alu.add)
        rsn = pc.tile([BSH, NROLES], dt.float32, tag="rsn")
        nc.vector.reciprocal(rsn[:], smn[:])
        role = pc.tile([BSH, NROLES * N], dt.float32, tag="role")
        nc.vector.tensor_tensor(role[:].rearrange("p (r n) -> p r n", n=N), exn_v,
                                rsn[:].unsqueeze(2).broadcast_to((BSH, NROLES, N)),
                                Alu.mult)
        nc.sync.dma_start(d_role, role[:].rearrange("p (r n) -> p r n", n=N))


def build(adj):
    nc = bacc.Bacc("TRN2", target_bir_lowering=False, debug=False,
                   num_devices=NCORES)
    with tile.TileContext(nc) as tc:
        _emit_program(nc, tc, adj)
    nc.compile()
    return nc


# ----------------------------------------------------------------------------
# public entry point
# ----------------------------------------------------------------------------

def kernel(**inputs):
    inputs = {k: np.asarray(v) for k, v in inputs.items()}
    adj = inputs["adj"]
    key = adj.tobytes()
    if key not in _cache:
        _cache[key] = build(adj)
    nc = _cache[key]

    np_dt = np.float32 if COMPUTE_DTYPE == "float32" else None
    # bfloat16 via ml_dtypes if available, else let DMA-side reinterpret handle it
    if np_dt is None:
        import ml_dtypes
        np_dt = ml_dtypes.bfloat16

    w = _prep_weights(inputs, np_dt)
    in_maps = []
    for c in range(NCORES):
        m = dict(w)
        m.update(_prep_core_inputs(inputs["z"], inputs["dz"], inputs["has"],
                                   c, np_dt))
        in_maps.append(m)
    res = run_bass_kernel_spmd(nc, in_maps, core_ids=list(range(NCORES)))
    outs = res.results
    instr = np.concatenate([outs[c]["out_instr"] for c in range(NCORES)], 0)
    role = np.concatenate([outs[c]["out_role"] for c in range(NCORES)], 0)
    value = np.concatenate([outs[c]["out_value"] for c in range(NCORES)], 0)
    return instr.astype(np.float32), role.astype(np.float32), value.astype(np.float32)
